# revision 4
# baseline (speedup 1.0000x reference)
"""Trainium2 Bass kernel for GQA causal self-attention with ALiBi.

Problem (hardcoded): B=4, T=2048, C=2048, n_head=16, n_kv=4, head_dim=128.
y = softmax(q k^T / sqrt(d) + alibi + causal) v, projected with Wo.

Sharding over 8 NeuronCores: data-parallel over the 4 batches x
tensor-parallel over 2 head groups (8 q heads / 2 kv heads per core,
keeping the GQA groups intact).  Each core computes a partial output
(its heads' contribution through its slice of Wo rows); the host sums
the two partials per batch.  No collectives needed.

Per-core kernel layout strategy (all matmuls in float32r = full-rate
fp32 on the PE with ~tf32 input rounding, fp32 PSUM accumulation):
  A) transpose x [T,C] -> xT [C,T] via PE-transpose (needed because the
     TensorE contracts over the partition dim), stage xT to DRAM.
  B) k^T [d, T] and v [T, d] projections (xT streamed back from DRAM).
  C) per 512-column i-block: q^T projection, then attention computed in
     the TRANSPOSED layout S^T[j, i] so that softmax normalization
     (sum over j) and P@V (contract over j) are both matmuls; ALiBi bias
     and the causal mask are a single DVE add of precomputed bias tiles
     (the exp(-slope*i) column factor cancels in softmax, so exponents
     stay small and no max-subtraction pass is needed); exp on ACT with
     a per-tile constant; denominator via ones-matmul; 1/denom is
     broadcast across partitions with a rank-1 PE matmul; finally the
     output projection accumulated over the 8 heads per 128-row tile.
Upper-triangle j-tiles are skipped entirely (causal halves the work).
"""

import sys

sys.path.insert(0, "/opt/trn_rl_repo")

import numpy as np

import bass_rust
import concourse.bass as bass
import concourse.mybir as mybir
import concourse.tile as tile

f32 = mybir.dt.float32
f32r = mybir.dt.float32r
AF = mybir.ActivationFunctionType

T = 2048
C = 2048
D = 128          # head dim
HQ = 8           # q heads per core
HKV = 2          # kv heads per core
NIB = 4          # i-blocks of 512 columns
IBW = 512
NJT = 16         # j-tiles of 128 rows
NCT = 16         # contraction (C) tiles of 128
SCALE = 1.0 / float(np.sqrt(D))
NBIAS = 10       # 2 slopes x (4 masked shifts + 1 unmasked)
NCONST = 33      # exp() per-tile additive constants


def hoist_excess_waits(nc, limit=1):
    """This walrus build rejects instructions carrying more than ~1 sync
    wait (fp32r matmul weight-load and CTRL structs overflow).  Keep the
    last wait on each instruction and hoist the rest onto NoOps placed
    immediately before it on the same engine queue."""
    n = 0
    for f in nc.m.functions:
        for bb in f.blocks:
            new_insts = []
            for inst in bb.instructions:
                si = inst.sync_info
                if si is not None and len(si.on_wait) > limit:
                    waits = list(si.on_wait)
                    extra, keep = waits[:-limit], waits[-limit:]
                    for k in range(0, len(extra), limit):
                        n += 1
                        nop = mybir.InstNoOp(
                            name=f"I-waitnop-{n}", engine=inst.engine)
                        nop.sync_info = bass_rust.SyncInfo(
                            on_wait=extra[k:k + limit], on_update=[])
                        new_insts.append(nop)
                    inst.sync_info = bass_rust.SyncInfo(
                        on_wait=keep, on_update=list(si.on_update))
                new_insts.append(inst)
            bb.instructions = new_insts
    return n


def build_nc():
    nc = bass.Bass("TRN2", target_bir_lowering=False, debug=False,
                   num_devices=1)

    X = nc.dram_tensor("x", [T, C], f32, kind="ExternalInput")
    WQ = nc.dram_tensor("Wq", [C, HQ * D], f32, kind="ExternalInput")
    WK = nc.dram_tensor("Wk", [C, HKV * D], f32, kind="ExternalInput")
    WV = nc.dram_tensor("Wv", [C, HKV * D], f32, kind="ExternalInput")
    WO = nc.dram_tensor("Wo", [HQ * D, C], f32, kind="ExternalInput")
    BIAST = nc.dram_tensor("BIAST", [128, NBIAS * IBW], f32,
                           kind="ExternalInput")
    CTAB = nc.dram_tensor("CTAB", [128, NCONST], f32, kind="ExternalInput")
    IDENT = nc.dram_tensor("IDENT", [128, 128], f32, kind="ExternalInput")
    ONES = nc.dram_tensor("ONES", [128, 1], f32, kind="ExternalInput")
    OUT = nc.dram_tensor("out", [T, C], f32, kind="ExternalOutput")

    with tile.TileContext(nc) as tc:
        with (
            tc.tile_pool(name="persist", bufs=1) as persist,
            tc.tile_pool(name="xTd", bufs=NCT, space="DRAM") as xTd,
        ):
            ident = persist.tile([128, 128], f32, tag="ident")
            nc.sync.dma_start(ident[:], IDENT.ap())
            ones_col = persist.tile([128, 1], f32r, tag="onc")
            nc.sync.dma_start(ones_col[:], ONES.ap().bitcast(f32r))
            ones_row = persist.tile([1, 128], f32r, tag="onr")
            nc.sync.dma_start(
                ones_row[:], ONES.ap().bitcast(f32r).rearrange("p o -> o p"))
            ctab = persist.tile([128, NCONST], f32, tag="ctab")
            nc.sync.dma_start(ctab[:], CTAB.ap())
            bias_sb = persist.tile([128, NBIAS * IBW], f32, tag="bias")
            nc.sync.dma_start(bias_sb[:], BIAST.ap())

            kT = [persist.tile([128, T], f32r, tag=f"kT{h}", name=f"kT{h}")
                  for h in range(HKV)]
            v_sb = [persist.tile([128, HKV * D], f32r, tag=f"v{t}", name=f"v{t}")
                    for t in range(NJT)]
            # DRAM staging for xT: 16 row tiles [128 (c), T (t)]
            xT_rows = [xTd.tile([128, T], f32r, name=f"xTr{i}") for i in range(NCT)]

            # ---------------- Phase A: x -> xT ----------------
            with (
                tc.tile_pool(name="xin", bufs=4) as xin,
                tc.tile_pool(name="xstg", bufs=4) as xstg,
                tc.tile_pool(name="psT", bufs=4, space="PSUM") as psT,
            ):
                for cb in range(4):  # c-blocks of 512
                    stg = [xstg.tile([128, T], f32r, tag=f"stg{r}", name=f"stg{cb}_{r}")
                           for r in range(4)]
                    for tt in range(NJT):
                        xt = xin.tile([128, 512], f32, tag="xin")
                        nc.sync.dma_start(
                            xt[:],
                            X.ap()[tt * 128:(tt + 1) * 128,
                                   cb * 512:(cb + 1) * 512])
                        for r in range(4):
                            ps = psT.tile([128, 128], f32, tag="pst")
                            nc.tensor.transpose(
                                ps[:], xt[:, r * 128:(r + 1) * 128], ident[:])
                            dst = stg[r][:, tt * 128:(tt + 1) * 128]
                            if (tt + r) % 2 == 0:
                                nc.scalar.activation(dst, ps[:], AF.Copy,
                                                     bias=0.0, scale=1.0)
                            else:
                                nc.vector.tensor_copy(dst, ps[:])
                    for r in range(4):
                        nc.sync.dma_start(xT_rows[cb * 4 + r][:], stg[r][:])

            # ---------------- Phase B1: kT projection ----------------
            with (
                tc.tile_pool(name="xrow", bufs=3) as xrow,
                tc.tile_pool(name="wkv", bufs=3) as wkv,
                tc.tile_pool(name="psK", bufs=8, space="PSUM") as psK,
            ):
                psk = [psK.tile([128, 512], f32, tag="psk", name=f"psk{i}")
                       for i in range(8)]
                for cc in range(NCT):
                    wk = wkv.tile([128, HKV * D], f32r, tag="wk")
                    nc.sync.dma_start(
                        wk[:],
                        WK.ap().bitcast(f32r)[cc * 128:(cc + 1) * 128, :])
                    xr = xrow.tile([128, T], f32r, tag="xrow")
                    nc.sync.dma_start(xr[:], xT_rows[cc][:])
                    for kvh in range(HKV):
                        for tch in range(4):
                            nc.tensor.matmul(
                                psk[kvh * 4 + tch][:],
                                wk[:, kvh * D:(kvh + 1) * D],
                                xr[:, tch * 512:(tch + 1) * 512],
                                start=(cc == 0), stop=(cc == NCT - 1))
                for kvh in range(HKV):
                    for tch in range(4):
                        nc.scalar.activation(
                            kT[kvh][:, tch * 512:(tch + 1) * 512],
                            psk[kvh * 4 + tch][:], AF.Copy,
                            bias=0.0, scale=1.0)

            # ---------------- Phase B2: v projection ----------------
            with (
                tc.tile_pool(name="xrow2", bufs=3) as xrow2,
                tc.tile_pool(name="wkv2", bufs=3) as wkv2,
                tc.tile_pool(name="psV", bufs=8, space="PSUM") as psV,
            ):
                for g in range(2):
                    psv = [psV.tile([128, HKV * D], f32, tag="psv",
                                    name=f"psv{g}_{i}") for i in range(8)]
                    for cc in range(NCT):
                        wv = wkv2.tile([128, HKV * D], f32r, tag="wv")
                        nc.sync.dma_start(
                            wv[:],
                            WV.ap().bitcast(f32r)[cc * 128:(cc + 1) * 128, :])
                        xr = xrow2.tile([128, T], f32r, tag="xrow2")
                        nc.sync.dma_start(xr[:], xT_rows[cc][:])
                        for tl in range(8):
                            t = g * 8 + tl
                            nc.tensor.matmul(
                                psv[tl][:],
                                xr[:, t * 128:(t + 1) * 128],
                                wv[:],
                                start=(cc == 0), stop=(cc == NCT - 1))
                    for tl in range(8):
                        nc.vector.tensor_copy(v_sb[g * 8 + tl][:], psv[tl][:])

            # ---------------- Phase C: per i-block ----------------
            with (
                tc.tile_pool(name="xts", bufs=18) as xts_pool,
                tc.tile_pool(name="wq", bufs=24) as wq_pool,
                tc.tile_pool(name="qt", bufs=8) as qt_pool,
                tc.tile_pool(name="ep", bufs=6) as e_pool,
                tc.tile_pool(name="yp", bufs=10) as y_pool,
                tc.tile_pool(name="wo", bufs=12) as wo_pool,
                tc.tile_pool(name="ob", bufs=4) as ob_pool,
                tc.tile_pool(name="rb", bufs=3) as rb_pool,
                tc.tile_pool(name="rc", bufs=3) as rc_pool,
                tc.tile_pool(name="psQ", bufs=2, space="PSUM") as psQ,
                tc.tile_pool(name="psS", bufs=2, space="PSUM") as psS,
                tc.tile_pool(name="psY", bufs=2, space="PSUM") as psY,
                tc.tile_pool(name="psD", bufs=2, space="PSUM") as psD,
            ):
                for ib in range(NIB):
                    nj = 4 * (ib + 1)
                    i0 = ib * IBW
                    # xT column slices for this i-block
                    xts = []
                    for cc in range(NCT):
                        xc = xts_pool.tile([128, IBW], f32r, tag="xts")
                        nc.sync.dma_start(
                            xc[:], xT_rows[cc][:, i0:i0 + IBW])
                        xts.append(xc)
                    # q^T projection (scale folded in)
                    qT = []
                    for h in range(HQ):
                        psq = psQ.tile([128, IBW], f32, tag="q")
                        for cc in range(NCT):
                            wq = wq_pool.tile([128, 128], f32r, tag="wq")
                            nc.sync.dma_start(
                                wq[:],
                                WQ.ap().bitcast(f32r)[
                                    cc * 128:(cc + 1) * 128,
                                    h * 128:(h + 1) * 128])
                            nc.tensor.matmul(
                                psq[:], wq[:], xts[cc][:],
                                start=(cc == 0), stop=(cc == NCT - 1))
                        qh = qt_pool.tile([128, IBW], f32r, tag="qt")
                        nc.scalar.activation(qh[:], psq[:], AF.Copy,
                                             bias=0.0, scale=SCALE)
                        qT.append(qh)
                    # attention
                    y_sb = []
                    for h in range(HQ):
                        kvh = h // 4
                        sl = h // 4  # slope index within core
                        psy = psY.tile([128, IBW], f32, tag="y")
                        psd = psD.tile([1, IBW], f32, tag="d")
                        for jt in range(nj):
                            pss = psS.tile([128, IBW], f32, tag="s")
                            nc.tensor.matmul(
                                pss[:],
                                kT[kvh][:, jt * 128:(jt + 1) * 128],
                                qT[h][:], start=True, stop=True)
                            s_rel = jt * 128 - i0
                            if s_rel < 0:
                                bidx = sl * 5 + 4
                                cidx = 1 + sl * 16 + (4 * ib - jt - 1)
                            else:
                                bidx = sl * 5 + s_rel // 128
                                cidx = 0
                            nc.vector.tensor_add(
                                pss[:], pss[:],
                                bias_sb[:, bidx * IBW:(bidx + 1) * IBW])
                            E = e_pool.tile([128, IBW], f32r, tag="E")
                            nc.scalar.activation(
                                E[:], pss[:], AF.Exp,
                                bias=ctab[:, cidx:cidx + 1], scale=1.0)
                            nc.tensor.matmul(
                                psd[:], ones_col[:], E[:],
                                start=(jt == 0), stop=(jt == nj - 1))
                            nc.tensor.matmul(
                                psy[:],
                                v_sb[jt][:, kvh * D:(kvh + 1) * D],
                                E[:],
                                start=(jt == 0), stop=(jt == nj - 1))
                        rc = rc_pool.tile([1, IBW], f32r, tag="rc")
                        with nc.allow_low_precision(
                                reason="f32r reciprocal for bcast matmul"):
                            nc.vector.reciprocal(rc[:], psd[:])
                        psb = psQ.tile([128, IBW], f32, tag="q")
                        nc.tensor.matmul(psb[:], ones_row[:], rc[:],
                                         start=True, stop=True)
                        rb = rb_pool.tile([128, IBW], f32, tag="rb")
                        nc.scalar.activation(rb[:], psb[:], AF.Copy,
                                             bias=0.0, scale=1.0)
                        yh = y_pool.tile([128, IBW], f32r, tag="ysb")
                        nc.vector.tensor_mul(yh[:], psy[:], rb[:])
                        y_sb.append(yh)
                    # output projection for this i-block's rows
                    for ncc in range(4):
                        wo_t = []
                        for h in range(HQ):
                            wo = wo_pool.tile([128, 512], f32r, tag="wo")
                            nc.sync.dma_start(
                                wo[:],
                                WO.ap().bitcast(f32r)[
                                    h * 128:(h + 1) * 128,
                                    ncc * 512:(ncc + 1) * 512])
                            wo_t.append(wo)
                        for ts in range(4):
                            pso = psS.tile([128, 512], f32, tag="s")
                            for h in range(HQ):
                                nc.tensor.matmul(
                                    pso[:],
                                    y_sb[h][:, ts * 128:(ts + 1) * 128],
                                    wo_t[h][:],
                                    start=(h == 0), stop=(h == HQ - 1))
                            ob = ob_pool.tile([128, 512], f32, tag="ob")
                            nc.scalar.activation(ob[:], pso[:], AF.Copy,
                                                 bias=0.0, scale=1.0)
                            nc.sync.dma_start(
                                OUT.ap()[i0 + ts * 128:i0 + (ts + 1) * 128,
                                         ncc * 512:(ncc + 1) * 512],
                                ob[:])

    hoist_excess_waits(nc)
    return nc


def host_tables(half):
    """Per-core bias/const tables. half=0 -> q heads 0-7, half=1 -> 8-15."""
    s_all = [2.0 ** (-(i + 1) / 2.0) for i in range(4)]
    slopes = [s_all[2 * half], s_all[2 * half + 1]]
    jr = np.arange(128, dtype=np.float64)[:, None]
    ir = np.arange(IBW, dtype=np.float64)[None, :]
    biast = np.zeros((128, NBIAS * IBW), np.float32)
    for sl in range(2):
        slope = slopes[sl]
        for k in range(4):
            s = 128 * k
            m = slope * (jr - ir + s)
            m[jr > ir - s] = -1e30
            biast[:, (sl * 5 + k) * IBW:(sl * 5 + k + 1) * IBW] = m
        biast[:, (sl * 5 + 4) * IBW:(sl * 5 + 5) * IBW] = slope * (jr - ir)
    ctab = np.zeros((128, NCONST), np.float32)
    for sl in range(2):
        for m in range(1, 17):
            ctab[:, 1 + sl * 16 + (m - 1)] = -slopes[sl] * 128.0 * m
    return biast, ctab


_RUNNER = None


def _get_runner():
    """Build the nc once and a cached jitted 8-core executor (mirrors
    bass2jax.run_bass_via_pjrt's multi-core path, but reusable across
    calls so repeated runs don't recompile the NEFF)."""
    global _RUNNER
    if _RUNNER is not None:
        return _RUNNER
    import jax
    from jax.experimental.shard_map import shard_map
    from jax.sharding import Mesh, PartitionSpec

    from concourse import bass2jax

    nc = build_nc()
    bass2jax.install_neuronx_cc_hook()

    partition_name = (nc.partition_id_tensor.name
                      if nc.partition_id_tensor else None)
    in_names = []
    out_names = []
    out_avals = []
    zero_outs = []
    for alloc in nc.m.functions[0].allocations:
        if not isinstance(alloc, mybir.MemoryLocationSet):
            continue
        if not alloc.memorylocations:
            continue
        name = alloc.memorylocations[0].name
        if alloc.kind == "ExternalInput":
            if name != partition_name:
                in_names.append(name)
        elif alloc.kind == "ExternalOutput":
            shape = tuple(alloc.tensor_shape)
            dtype = mybir.dt.np(alloc.dtype)
            out_names.append(name)
            out_avals.append(jax.core.ShapedArray(shape, dtype))
            zero_outs.append(np.zeros(shape, dtype))
    n_params = len(in_names)
    n_outs = len(out_names)
    all_names = in_names + out_names
    if partition_name is not None:
        all_names = all_names + [partition_name]

    def _body(*args):
        operands = list(args)
        if partition_name is not None:
            operands.append(bass2jax.partition_id_tensor())
        outs = bass2jax._bass_exec_p.bind(
            *operands,
            out_avals=tuple(out_avals),
            in_names=tuple(all_names),
            out_names=tuple(out_names),
            lowering_input_output_aliases=(),
            sim_require_finite=True,
            sim_require_nnan=True,
            nc=nc,
        )
        return tuple(outs)

    n_cores = 8
    devices = jax.devices()[:n_cores]
    mesh = Mesh(np.asarray(devices), ("core",))
    in_specs = (PartitionSpec("core"),) * (n_params + n_outs)
    out_specs = (PartitionSpec("core"),) * n_outs
    donate = tuple(range(n_params, n_params + n_outs))
    sharded = jax.jit(
        shard_map(_body, mesh=mesh, in_specs=in_specs, out_specs=out_specs,
                  check_rep=False),
        donate_argnums=donate, keep_unused=True,
    )

    def run(in_maps):
        per_core = [[np.asarray(m[name]) for name in in_names]
                    for m in in_maps]
        concat_in = [
            np.concatenate([per_core[c][i] for c in range(n_cores)], axis=0)
            for i in range(n_params)
        ]
        concat_zeros = [
            np.zeros((n_cores * z.shape[0], *z.shape[1:]), z.dtype)
            for z in zero_outs
        ]
        out_arrs = sharded(*concat_in, *concat_zeros)
        return [
            {name: np.asarray(out_arrs[i]).reshape(
                n_cores, *out_avals[i].shape)[c]
             for i, name in enumerate(out_names)}
            for c in range(n_cores)
        ]

    _RUNNER = (run, in_names)
    return _RUNNER


def make_in_maps(x, Wq, Wk, Wv, Wo):
    x = np.ascontiguousarray(np.asarray(x, dtype=np.float32))
    Wq = np.ascontiguousarray(np.asarray(Wq, dtype=np.float32))
    Wk = np.ascontiguousarray(np.asarray(Wk, dtype=np.float32))
    Wv = np.ascontiguousarray(np.asarray(Wv, dtype=np.float32))
    Wo = np.ascontiguousarray(np.asarray(Wo, dtype=np.float32))
    ident = np.eye(128, dtype=np.float32)
    ones = np.ones((128, 1), np.float32)
    tables = [host_tables(0), host_tables(1)]
    in_maps = []
    for core in range(8):
        b, half = core // 2, core % 2
        biast, ctab = tables[half]
        in_maps.append({
            "x": x[b],
            "Wq": np.ascontiguousarray(
                Wq[:, half * HQ * D:(half + 1) * HQ * D]),
            "Wk": np.ascontiguousarray(
                Wk[:, half * HKV * D:(half + 1) * HKV * D]),
            "Wv": np.ascontiguousarray(
                Wv[:, half * HKV * D:(half + 1) * HKV * D]),
            "Wo": np.ascontiguousarray(
                Wo[half * HQ * D:(half + 1) * HQ * D, :]),
            "BIAST": biast,
            "CTAB": ctab,
            "IDENT": ident,
            "ONES": ones,
        })
    return in_maps


def kernel(x, Wq, Wk, Wv, Wo):
    run, _ = _get_runner()
    in_maps = make_in_maps(x, Wq, Wk, Wv, Wo)
    results = run(in_maps)
    out = np.empty((4, T, C), np.float32)
    for b in range(4):
        out[b] = results[2 * b]["out"] + results[2 * b + 1]["out"]
    return out


# revision 5
# speedup vs baseline: 178.7696x; 178.7696x over previous
"""Trainium2 Bass kernel for GQA causal self-attention with ALiBi.

Problem (hardcoded): B=4, T=2048, C=2048, n_head=16, n_kv=4, head_dim=128.
y = softmax(q k^T / sqrt(d) + alibi + causal) v, projected with Wo.

Sharding over 8 NeuronCores: data-parallel over the 4 batches x
tensor-parallel over 2 head groups (8 q heads / 2 kv heads per core,
keeping the GQA groups intact).  Each core computes a partial output
(its heads' contribution through its slice of Wo rows); the host sums
the two partials per batch.  No collectives needed.

Per-core kernel layout strategy (all matmuls in float32r = full-rate
fp32 on the PE with ~tf32 input rounding, fp32 PSUM accumulation):
  A) transpose x [T,C] -> xT [C,T] via PE-transpose (needed because the
     TensorE contracts over the partition dim), stage xT to DRAM.
  B) k^T [d, T] and v [T, d] projections (xT streamed back from DRAM).
  C) per 512-column i-block: q^T projection, then attention computed in
     the TRANSPOSED layout S^T[j, i] so that softmax normalization
     (sum over j) and P@V (contract over j) are both matmuls; ALiBi bias
     and the causal mask are a single DVE add of precomputed bias tiles
     (the exp(-slope*i) column factor cancels in softmax, so exponents
     stay small and no max-subtraction pass is needed); exp on ACT with
     a per-tile constant; denominator via ones-matmul; 1/denom is
     broadcast across partitions with a rank-1 PE matmul; finally the
     output projection accumulated over the 8 heads per 128-row tile.
Upper-triangle j-tiles are skipped entirely (causal halves the work).
"""

import sys

sys.path.insert(0, "/opt/trn_rl_repo")

import numpy as np

import bass_rust
import concourse.bass as bass
import concourse.mybir as mybir
import concourse.tile as tile

f32 = mybir.dt.float32
f32r = mybir.dt.float32r
AF = mybir.ActivationFunctionType

T = 2048
C = 2048
D = 128          # head dim
HQ = 8           # q heads per core
HKV = 2          # kv heads per core
NIB = 4          # i-blocks of 512 columns
IBW = 512
NJT = 16         # j-tiles of 128 rows
NCT = 16         # contraction (C) tiles of 128
SCALE = 1.0 / float(np.sqrt(D))
NBIAS = 10       # 2 slopes x (4 masked shifts + 1 unmasked)
NCONST = 33      # exp() per-tile additive constants


def hoist_excess_waits(nc, limit=1):
    """This walrus build rejects instructions carrying more than ~1 sync
    wait (fp32r matmul weight-load and CTRL structs overflow).  Keep the
    last wait on each instruction and hoist the rest onto NoOps placed
    immediately before it on the same engine queue."""
    n = 0
    for f in nc.m.functions:
        for bb in f.blocks:
            new_insts = []
            for inst in bb.instructions:
                si = inst.sync_info
                if si is not None and len(si.on_wait) > limit:
                    waits = list(si.on_wait)
                    extra, keep = waits[:-limit], waits[-limit:]
                    for k in range(0, len(extra), limit):
                        n += 1
                        nop = mybir.InstNoOp(
                            name=f"I-waitnop-{n}", engine=inst.engine)
                        nop.sync_info = bass_rust.SyncInfo(
                            on_wait=extra[k:k + limit], on_update=[])
                        new_insts.append(nop)
                    inst.sync_info = bass_rust.SyncInfo(
                        on_wait=keep, on_update=list(si.on_update))
                new_insts.append(inst)
            bb.instructions = new_insts
    return n


def build_nc():
    nc = bass.Bass("TRN2", target_bir_lowering=False, debug=False,
                   num_devices=1)

    X = nc.dram_tensor("x", [T, C], f32, kind="ExternalInput")
    WQ = nc.dram_tensor("Wq", [C, HQ * D], f32, kind="ExternalInput")
    WK = nc.dram_tensor("Wk", [C, HKV * D], f32, kind="ExternalInput")
    WV = nc.dram_tensor("Wv", [C, HKV * D], f32, kind="ExternalInput")
    WO = nc.dram_tensor("Wo", [HQ * D, C], f32, kind="ExternalInput")
    BIAST = nc.dram_tensor("BIAST", [128, NBIAS * IBW], f32,
                           kind="ExternalInput")
    CTAB = nc.dram_tensor("CTAB", [128, NCONST], f32, kind="ExternalInput")
    IDENT = nc.dram_tensor("IDENT", [128, 128], f32, kind="ExternalInput")
    ONES = nc.dram_tensor("ONES", [128, 1], f32, kind="ExternalInput")
    OUT = nc.dram_tensor("out", [T, C], f32, kind="ExternalOutput")

    with tile.TileContext(nc) as tc:
        with (
            tc.tile_pool(name="persist", bufs=1) as persist,
            tc.tile_pool(name="xTd", bufs=NCT, space="DRAM") as xTd,
        ):
            ident = persist.tile([128, 128], f32, tag="ident")
            nc.sync.dma_start(ident[:], IDENT.ap())
            ones_col = persist.tile([128, 1], f32r, tag="onc")
            nc.sync.dma_start(ones_col[:], ONES.ap().bitcast(f32r))
            ones_row = persist.tile([1, 128], f32r, tag="onr")
            nc.sync.dma_start(
                ones_row[:], ONES.ap().bitcast(f32r).rearrange("p o -> o p"))
            ctab = persist.tile([128, NCONST], f32, tag="ctab")
            nc.sync.dma_start(ctab[:], CTAB.ap())
            bias_sb = persist.tile([128, NBIAS * IBW], f32, tag="bias")
            nc.sync.dma_start(bias_sb[:], BIAST.ap())

            kT = [persist.tile([128, T], f32r, tag=f"kT{h}", name=f"kT{h}")
                  for h in range(HKV)]
            v_sb = [persist.tile([128, HKV * D], f32r, tag=f"v{t}", name=f"v{t}")
                    for t in range(NJT)]
            # DRAM staging for xT: 16 row tiles [128 (c), T (t)]
            xT_rows = [xTd.tile([128, T], f32r, name=f"xTr{i}") for i in range(NCT)]

            # ---------------- Phase A: x -> xT ----------------
            with (
                tc.tile_pool(name="xin", bufs=4) as xin,
                tc.tile_pool(name="xstg", bufs=4) as xstg,
                tc.tile_pool(name="psT", bufs=4, space="PSUM") as psT,
            ):
                for cb in range(4):  # c-blocks of 512
                    stg = [xstg.tile([128, T], f32r, tag=f"stg{r}", name=f"stg{cb}_{r}")
                           for r in range(4)]
                    for tt in range(NJT):
                        xt = xin.tile([128, 512], f32, tag="xin")
                        nc.sync.dma_start(
                            xt[:],
                            X.ap()[tt * 128:(tt + 1) * 128,
                                   cb * 512:(cb + 1) * 512])
                        for r in range(4):
                            ps = psT.tile([128, 128], f32, tag="pst")
                            nc.tensor.transpose(
                                ps[:], xt[:, r * 128:(r + 1) * 128], ident[:])
                            dst = stg[r][:, tt * 128:(tt + 1) * 128]
                            if (tt + r) % 2 == 0:
                                nc.scalar.activation(dst, ps[:], AF.Copy,
                                                     bias=0.0, scale=1.0)
                            else:
                                nc.vector.tensor_copy(dst, ps[:])
                    for r in range(4):
                        nc.sync.dma_start(xT_rows[cb * 4 + r][:], stg[r][:])

            # ---------------- Phase B1: kT projection ----------------
            with (
                tc.tile_pool(name="xrow", bufs=3) as xrow,
                tc.tile_pool(name="wkv", bufs=3) as wkv,
                tc.tile_pool(name="psK", bufs=8, space="PSUM") as psK,
            ):
                psk = [psK.tile([128, 512], f32, tag="psk", name=f"psk{i}")
                       for i in range(8)]
                for cc in range(NCT):
                    wk = wkv.tile([128, HKV * D], f32r, tag="wk")
                    nc.sync.dma_start(
                        wk[:],
                        WK.ap().bitcast(f32r)[cc * 128:(cc + 1) * 128, :])
                    xr = xrow.tile([128, T], f32r, tag="xrow")
                    nc.sync.dma_start(xr[:], xT_rows[cc][:])
                    for kvh in range(HKV):
                        for tch in range(4):
                            nc.tensor.matmul(
                                psk[kvh * 4 + tch][:],
                                wk[:, kvh * D:(kvh + 1) * D],
                                xr[:, tch * 512:(tch + 1) * 512],
                                start=(cc == 0), stop=(cc == NCT - 1))
                for kvh in range(HKV):
                    for tch in range(4):
                        nc.scalar.activation(
                            kT[kvh][:, tch * 512:(tch + 1) * 512],
                            psk[kvh * 4 + tch][:], AF.Copy,
                            bias=0.0, scale=1.0)

            # ---------------- Phase B2: v projection ----------------
            with (
                tc.tile_pool(name="xrow2", bufs=3) as xrow2,
                tc.tile_pool(name="wkv2", bufs=3) as wkv2,
                tc.tile_pool(name="psV", bufs=8, space="PSUM") as psV,
            ):
                for g in range(2):
                    psv = [psV.tile([128, HKV * D], f32, tag="psv",
                                    name=f"psv{g}_{i}") for i in range(8)]
                    for cc in range(NCT):
                        wv = wkv2.tile([128, HKV * D], f32r, tag="wv")
                        nc.sync.dma_start(
                            wv[:],
                            WV.ap().bitcast(f32r)[cc * 128:(cc + 1) * 128, :])
                        xr = xrow2.tile([128, T], f32r, tag="xrow2")
                        nc.sync.dma_start(xr[:], xT_rows[cc][:])
                        for tl in range(8):
                            t = g * 8 + tl
                            nc.tensor.matmul(
                                psv[tl][:],
                                xr[:, t * 128:(t + 1) * 128],
                                wv[:],
                                start=(cc == 0), stop=(cc == NCT - 1))
                    for tl in range(8):
                        nc.vector.tensor_copy(v_sb[g * 8 + tl][:], psv[tl][:])

            # ---------------- Phase C: per i-block ----------------
            with (
                tc.tile_pool(name="xts", bufs=18) as xts_pool,
                tc.tile_pool(name="wq", bufs=24) as wq_pool,
                tc.tile_pool(name="qt", bufs=8) as qt_pool,
                tc.tile_pool(name="ep", bufs=6) as e_pool,
                tc.tile_pool(name="yp", bufs=10) as y_pool,
                tc.tile_pool(name="wo", bufs=12) as wo_pool,
                tc.tile_pool(name="ob", bufs=4) as ob_pool,
                tc.tile_pool(name="rb", bufs=3) as rb_pool,
                tc.tile_pool(name="rc", bufs=3) as rc_pool,
                tc.tile_pool(name="psQ", bufs=2, space="PSUM") as psQ,
                tc.tile_pool(name="psS", bufs=2, space="PSUM") as psS,
                tc.tile_pool(name="psY", bufs=2, space="PSUM") as psY,
                tc.tile_pool(name="psD", bufs=2, space="PSUM") as psD,
            ):
                for ib in range(NIB):
                    nj = 4 * (ib + 1)
                    i0 = ib * IBW
                    # xT column slices for this i-block
                    xts = []
                    for cc in range(NCT):
                        xc = xts_pool.tile([128, IBW], f32r, tag="xts")
                        nc.sync.dma_start(
                            xc[:], xT_rows[cc][:, i0:i0 + IBW])
                        xts.append(xc)
                    # q^T projection (scale folded in)
                    qT = []
                    for h in range(HQ):
                        psq = psQ.tile([128, IBW], f32, tag="q")
                        for cc in range(NCT):
                            wq = wq_pool.tile([128, 128], f32r, tag="wq")
                            nc.sync.dma_start(
                                wq[:],
                                WQ.ap().bitcast(f32r)[
                                    cc * 128:(cc + 1) * 128,
                                    h * 128:(h + 1) * 128])
                            nc.tensor.matmul(
                                psq[:], wq[:], xts[cc][:],
                                start=(cc == 0), stop=(cc == NCT - 1))
                        qh = qt_pool.tile([128, IBW], f32r, tag="qt")
                        nc.scalar.activation(qh[:], psq[:], AF.Copy,
                                             bias=0.0, scale=SCALE)
                        qT.append(qh)
                    # attention
                    y_sb = []
                    for h in range(HQ):
                        kvh = h // 4
                        sl = h // 4  # slope index within core
                        psy = psY.tile([128, IBW], f32, tag="y")
                        psd = psD.tile([1, IBW], f32, tag="d")
                        for jt in range(nj):
                            pss = psS.tile([128, IBW], f32, tag="s")
                            nc.tensor.matmul(
                                pss[:],
                                kT[kvh][:, jt * 128:(jt + 1) * 128],
                                qT[h][:], start=True, stop=True)
                            s_rel = jt * 128 - i0
                            if s_rel < 0:
                                bidx = sl * 5 + 4
                                cidx = 1 + sl * 16 + (4 * ib - jt - 1)
                            else:
                                bidx = sl * 5 + s_rel // 128
                                cidx = 0
                            nc.vector.tensor_add(
                                pss[:], pss[:],
                                bias_sb[:, bidx * IBW:(bidx + 1) * IBW])
                            E = e_pool.tile([128, IBW], f32r, tag="E")
                            nc.scalar.activation(
                                E[:], pss[:], AF.Exp,
                                bias=ctab[:, cidx:cidx + 1], scale=1.0)
                            nc.tensor.matmul(
                                psd[:], ones_col[:], E[:],
                                start=(jt == 0), stop=(jt == nj - 1))
                            nc.tensor.matmul(
                                psy[:],
                                v_sb[jt][:, kvh * D:(kvh + 1) * D],
                                E[:],
                                start=(jt == 0), stop=(jt == nj - 1))
                        rc = rc_pool.tile([1, IBW], f32r, tag="rc")
                        with nc.allow_low_precision(
                                reason="f32r reciprocal for bcast matmul"):
                            nc.vector.reciprocal(rc[:], psd[:])
                        psb = psQ.tile([128, IBW], f32, tag="q")
                        nc.tensor.matmul(psb[:], ones_row[:], rc[:],
                                         start=True, stop=True)
                        rb = rb_pool.tile([128, IBW], f32, tag="rb")
                        nc.scalar.activation(rb[:], psb[:], AF.Copy,
                                             bias=0.0, scale=1.0)
                        yh = y_pool.tile([128, IBW], f32r, tag="ysb")
                        nc.vector.tensor_mul(yh[:], psy[:], rb[:])
                        y_sb.append(yh)
                    # output projection for this i-block's rows
                    for ncc in range(4):
                        wo_t = []
                        for h in range(HQ):
                            wo = wo_pool.tile([128, 512], f32r, tag="wo")
                            nc.sync.dma_start(
                                wo[:],
                                WO.ap().bitcast(f32r)[
                                    h * 128:(h + 1) * 128,
                                    ncc * 512:(ncc + 1) * 512])
                            wo_t.append(wo)
                        for ts in range(4):
                            pso = psS.tile([128, 512], f32, tag="s")
                            for h in range(HQ):
                                nc.tensor.matmul(
                                    pso[:],
                                    y_sb[h][:, ts * 128:(ts + 1) * 128],
                                    wo_t[h][:],
                                    start=(h == 0), stop=(h == HQ - 1))
                            ob = ob_pool.tile([128, 512], f32, tag="ob")
                            nc.scalar.activation(ob[:], pso[:], AF.Copy,
                                                 bias=0.0, scale=1.0)
                            nc.sync.dma_start(
                                OUT.ap()[i0 + ts * 128:i0 + (ts + 1) * 128,
                                         ncc * 512:(ncc + 1) * 512],
                                ob[:])

    hoist_excess_waits(nc)
    return nc


def host_tables(half):
    """Per-core bias/const tables. half=0 -> q heads 0-7, half=1 -> 8-15."""
    s_all = [2.0 ** (-(i + 1) / 2.0) for i in range(4)]
    slopes = [s_all[2 * half], s_all[2 * half + 1]]
    jr = np.arange(128, dtype=np.float64)[:, None]
    ir = np.arange(IBW, dtype=np.float64)[None, :]
    biast = np.zeros((128, NBIAS * IBW), np.float32)
    for sl in range(2):
        slope = slopes[sl]
        for k in range(4):
            s = 128 * k
            m = slope * (jr - ir + s)
            m[jr > ir - s] = -1e30
            biast[:, (sl * 5 + k) * IBW:(sl * 5 + k + 1) * IBW] = m
        biast[:, (sl * 5 + 4) * IBW:(sl * 5 + 5) * IBW] = slope * (jr - ir)
    ctab = np.zeros((128, NCONST), np.float32)
    for sl in range(2):
        for m in range(1, 17):
            ctab[:, 1 + sl * 16 + (m - 1)] = -slopes[sl] * 128.0 * m
    return biast, ctab


_RUNNER = None


def _get_runner():
    """Build the nc once and a cached jitted 8-core executor (mirrors
    bass2jax.run_bass_via_pjrt's multi-core path, but reusable across
    calls so repeated runs don't recompile the NEFF)."""
    global _RUNNER
    if _RUNNER is not None:
        return _RUNNER
    import jax
    from jax.experimental.shard_map import shard_map
    from jax.sharding import Mesh, PartitionSpec

    from concourse import bass2jax

    nc = build_nc()
    bass2jax.install_neuronx_cc_hook()

    partition_name = (nc.partition_id_tensor.name
                      if nc.partition_id_tensor else None)
    in_names = []
    out_names = []
    out_avals = []
    zero_outs = []
    for alloc in nc.m.functions[0].allocations:
        if not isinstance(alloc, mybir.MemoryLocationSet):
            continue
        if not alloc.memorylocations:
            continue
        name = alloc.memorylocations[0].name
        if alloc.kind == "ExternalInput":
            if name != partition_name:
                in_names.append(name)
        elif alloc.kind == "ExternalOutput":
            shape = tuple(alloc.tensor_shape)
            dtype = mybir.dt.np(alloc.dtype)
            out_names.append(name)
            out_avals.append(jax.core.ShapedArray(shape, dtype))
            zero_outs.append(np.zeros(shape, dtype))
    n_params = len(in_names)
    n_outs = len(out_names)
    all_names = in_names + out_names
    if partition_name is not None:
        all_names = all_names + [partition_name]

    def _body(*args):
        operands = list(args)
        if partition_name is not None:
            operands.append(bass2jax.partition_id_tensor())
        outs = bass2jax._bass_exec_p.bind(
            *operands,
            out_avals=tuple(out_avals),
            in_names=tuple(all_names),
            out_names=tuple(out_names),
            lowering_input_output_aliases=(),
            sim_require_finite=True,
            sim_require_nnan=True,
            nc=nc,
        )
        return tuple(outs)

    n_cores = 8
    devices = jax.devices()[:n_cores]
    mesh = Mesh(np.asarray(devices), ("core",))
    in_specs = (PartitionSpec("core"),) * (n_params + n_outs)
    out_specs = (PartitionSpec("core"),) * n_outs
    donate = tuple(range(n_params, n_params + n_outs))
    sharded = jax.jit(
        shard_map(_body, mesh=mesh, in_specs=in_specs, out_specs=out_specs,
                  check_rep=False),
        donate_argnums=donate, keep_unused=True,
    )

    def run(in_maps):
        per_core = [[np.asarray(m[name]) for name in in_names]
                    for m in in_maps]
        concat_in = [
            np.concatenate([per_core[c][i] for c in range(n_cores)], axis=0)
            for i in range(n_params)
        ]
        concat_zeros = [
            np.zeros((n_cores * z.shape[0], *z.shape[1:]), z.dtype)
            for z in zero_outs
        ]
        out_arrs = sharded(*concat_in, *concat_zeros)
        return [
            {name: np.asarray(out_arrs[i]).reshape(
                n_cores, *out_avals[i].shape)[c]
             for i, name in enumerate(out_names)}
            for c in range(n_cores)
        ]

    def make_chained(n_chain):
        def _chain(*args):
            ins = list(args[:n_params])
            outs = list(args[n_params:])
            for _ in range(n_chain):
                operands = ins + outs
                if partition_name is not None:
                    operands.append(bass2jax.partition_id_tensor())
                outs = list(bass2jax._bass_exec_p.bind(
                    *operands,
                    out_avals=tuple(out_avals),
                    in_names=tuple(all_names),
                    out_names=tuple(out_names),
                    lowering_input_output_aliases=(),
                    sim_require_finite=True,
                    sim_require_nnan=True,
                    nc=nc,
                ))
            return tuple(outs)

        return jax.jit(
            shard_map(_chain, mesh=mesh, in_specs=in_specs,
                      out_specs=out_specs, check_rep=False),
            donate_argnums=donate, keep_unused=True,
        )

    def prep_args(in_maps):
        per_core = [[np.asarray(m[name]) for name in in_names]
                    for m in in_maps]
        concat_in = [
            np.concatenate([per_core[c][i] for c in range(n_cores)], axis=0)
            for i in range(n_params)
        ]
        concat_zeros = [
            np.zeros((n_cores * z.shape[0], *z.shape[1:]), z.dtype)
            for z in zero_outs
        ]
        return concat_in, concat_zeros

    _RUNNER = (run, in_names, make_chained, prep_args)
    return _RUNNER


def make_in_maps(x, Wq, Wk, Wv, Wo):
    x = np.ascontiguousarray(np.asarray(x, dtype=np.float32))
    Wq = np.ascontiguousarray(np.asarray(Wq, dtype=np.float32))
    Wk = np.ascontiguousarray(np.asarray(Wk, dtype=np.float32))
    Wv = np.ascontiguousarray(np.asarray(Wv, dtype=np.float32))
    Wo = np.ascontiguousarray(np.asarray(Wo, dtype=np.float32))
    ident = np.eye(128, dtype=np.float32)
    ones = np.ones((128, 1), np.float32)
    tables = [host_tables(0), host_tables(1)]
    in_maps = []
    for core in range(8):
        b, half = core // 2, core % 2
        biast, ctab = tables[half]
        in_maps.append({
            "x": x[b],
            "Wq": np.ascontiguousarray(
                Wq[:, half * HQ * D:(half + 1) * HQ * D]),
            "Wk": np.ascontiguousarray(
                Wk[:, half * HKV * D:(half + 1) * HKV * D]),
            "Wv": np.ascontiguousarray(
                Wv[:, half * HKV * D:(half + 1) * HKV * D]),
            "Wo": np.ascontiguousarray(
                Wo[half * HQ * D:(half + 1) * HQ * D, :]),
            "BIAST": biast,
            "CTAB": ctab,
            "IDENT": ident,
            "ONES": ones,
        })
    return in_maps


def kernel(x, Wq, Wk, Wv, Wo):
    run = _get_runner()[0]
    in_maps = make_in_maps(x, Wq, Wk, Wv, Wo)
    results = run(in_maps)
    out = np.empty((4, T, C), np.float32)
    for b in range(4):
        out[b] = results[2 * b]["out"] + results[2 * b + 1]["out"]
    return out
